# revision 23
# baseline (speedup 1.0000x reference)
"""Trainium2 Bass kernel for nn_CP_L3_sparse_outer.

Math (per batch row b):
    s2[b] = sum_d U2[d] * z[b, d]
    s3[b] = sum_d U3[d] * z[b, d]
    out[b, o] = (s2[b] * s3[b]) * sum_d (U1[d] * z[b, d]) * W[o, d] + bias[o]

Sharding: data-parallel over batch B=8192 across 8 NeuronCores
(B_loc = 1024 rows per core); W / U1 / U2 / U3 / bias replicated.

Per-core plan (f32 storage, main matmuls in float32r = 1 cyc/row at N=512):
  A. Load z row-tiles, stage through a DVE copy (collapses every PE
     transpose's waits onto the DVE semaphore), transpose 128x128 chunks on
     TensorE into resident ztbig = z.T [128 d_in, k(32) * 1024 b].
     Transposes write 4-chunk groups into one full PSUM bank so the bank WAR
     is dominated by the (newer) DVE data wait -> 1 sem wait per matmul
     (walrus allows only one on Matmult/DMACopy).
  B. s2/s3 via PE matmuls: psum[128 b, 2] += zT_chunk.T @ U23_chunk.
  C. c = s2*s3 -> per-tile PE transpose [128,1]->[1,128] -> ones[1,128]
     outer-product matmul -> cbcast [128, 1024] (c broadcast on partitions).
  D. zT = (zT * U1_per_partition) * cbcast in place (one DVE op per chunk),
     rounding to f32r on the write.
  E. Main matmul, output-transposed: per o-tile (32): psum [128 o, 512 b] x2
     accumulate over k with lhsT = W.T chunk (streamed), rhs = zT (resident);
     evict + bias via DVE tensor_scalar; transpose back on TensorE; batched
     SWDGE store to out[b, o].

Big/repeated DMAs go through SWDGE (gpsimd) whose ucode tolerates multiple
sem waits; HWDGE (sync) handles only the one-shot constant loads.
Host-side prep is layout-only: WT = W.T contiguous, U23 = stack(U2, U3).
"""

import os
import sys

import numpy as np

if "/opt/trn_rl_repo" not in sys.path:
    sys.path.insert(0, "/opt/trn_rl_repo")

import concourse.bass as bass
from concourse import bacc
import concourse.mybir as mybir
import concourse.tile as tile
from concourse.masks import make_identity

P = 128
D = 4096
O = 4096
B = 8192
NCORES = 8
BLOC = B // NCORES          # 1024 batch rows per core
KC = D // P                 # 32 contraction chunks
BT = BLOC // P              # 8 batch tiles of 128
OT = O // P                 # 32 output tiles of 128
NH = BLOC // 512            # 2 rhs halves of 512
QW = 1024                   # z row-segment width for phase A staging
NQ = D // QW                # 4 segments per batch tile
F32 = mybir.dt.float32
F32R = mybir.dt.float32r
MULT = mybir.AluOpType.mult


def build_nc() -> bass.Bass:
    nc = bacc.Bacc(trn_type="TRN2")

    z_d = nc.dram_tensor("z", [BLOC, D], F32, kind="ExternalInput")
    wt_d = nc.dram_tensor("wt", [D, O], F32R, kind="ExternalInput")
    u1_d = nc.dram_tensor("u1", [D], F32, kind="ExternalInput")
    u23_d = nc.dram_tensor("u23", [D, 2], F32, kind="ExternalInput")
    bias_d = nc.dram_tensor("bias", [O], F32, kind="ExternalInput")
    out_d = nc.dram_tensor("out", [BLOC, O], F32, kind="ExternalOutput")

    with tile.TileContext(nc) as tc:
        with (
            tc.tile_pool(name="const", bufs=1) as const,
            tc.tile_pool(name="ztp", bufs=1) as ztp,
            tc.tile_pool(name="znat", bufs=2) as znatp,
            tc.tile_pool(name="part", bufs=3) as partp,
            tc.tile_pool(name="wslab", bufs=2) as wslabp,
            tc.tile_pool(name="outT", bufs=1) as outTp,
            tc.tile_pool(name="onat", bufs=2) as onatp,
            tc.tile_pool(name="pmain", bufs=4, space="PSUM") as pmain,
            tc.tile_pool(name="ptr", bufs=2, space="PSUM") as ptr,
            tc.tile_pool(name="pmisc", bufs=2, space="PSUM") as pmisc,
        ):
            # ---- constants (one-shot HWDGE loads) ----
            identity = const.tile([P, P], F32)
            make_identity(nc, identity)
            ones1 = const.tile([1, P], F32)
            nc.vector.memset(ones1[:], 1.0)
            u1sb = const.tile([P, KC], F32)
            nc.sync.dma_start(u1sb[:], u1_d[:].rearrange("(k p) -> p k", p=P))
            biassb = const.tile([P, OT], F32)
            nc.sync.dma_start(biassb[:], bias_d[:].rearrange("(a p) -> p a", p=P))
            csb = const.tile([P, BT], F32)
            s2col = const.tile([P, BT], F32)
            s3col = const.tile([P, BT], F32)
            nc.vector.memset(s2col[:], 0.0)
            nc.vector.memset(s3col[:], 0.0)
            crow = const.tile([1, BLOC], F32)
            cbcast = const.tile([P, BLOC], F32)
            u2bcast = const.tile([P, QW], F32)
            u3bcast = const.tile([P, QW], F32)
            urow = const.tile([1, QW], F32)
            scratch = const.tile([P, QW], F32)

            # warm-up transpose (absorbs identity readiness once)
            ptw = ptr.tile([P, 512], F32, name="pt", tag="pt")
            nc.tensor.transpose(ptw[:, 0:P], identity[:], identity[:])

            # zT resident: [128 d_in, k * BLOC + b]
            ztbig = ztp.tile([P, KC * BLOC], F32R)

            # ---- phase A: transpose z into ztbig; s2/s3 on DVE ----
            # q-major so the U2/U3 broadcast tiles are built once per q.
            ADD = mybir.AluOpType.add
            for q in range(NQ):
                for u, ub in enumerate([u2bcast, u3bcast]):
                    nc.sync.dma_start(
                        urow[0:1, :],
                        u23_d[:][q * QW : (q + 1) * QW, u : u + 1].rearrange(
                            "d u -> u d"
                        ),
                    )
                    for j in range(QW // 512):
                        pu = pmisc.tile([P, 512], F32, name="pu", tag="pmisc")
                        nc.tensor.matmul(
                            pu[:], ones1[:],
                            urow[0:1, j * 512 : (j + 1) * 512],
                            start=True, stop=True,
                        )
                        nc.vector.tensor_copy(
                            ub[:, j * 512 : (j + 1) * 512], pu[:]
                        )
                for bt in range(BT):
                    znat = znatp.tile([P, QW], F32, name="znat")
                    nc.gpsimd.dma_start(
                        znat[:],
                        z_d[:][bt * P : (bt + 1) * P, q * QW : (q + 1) * QW],
                    )
                    part = partp.tile([P, 2], F32, name="part")
                    nc.vector.scalar_tensor_tensor(
                        scratch[:], znat[:], 1.0, u2bcast[:],
                        MULT, MULT, accum_out=part[:, 0:1],
                    )
                    nc.vector.scalar_tensor_tensor(
                        scratch[:], znat[:], 1.0, u3bcast[:],
                        MULT, MULT, accum_out=part[:, 1:2],
                    )
                    nc.vector.tensor_add(
                        s2col[:, bt : bt + 1], s2col[:, bt : bt + 1], part[:, 0:1]
                    )
                    nc.vector.tensor_add(
                        s3col[:, bt : bt + 1], s3col[:, bt : bt + 1], part[:, 1:2]
                    )
                    for g in range(QW // 512):
                        pt = ptr.tile([P, 512], F32, name="pt", tag="pt")
                        for i in range(4):
                            nc.tensor.transpose(
                                pt[:, i * P : (i + 1) * P],
                                znat[:, (g * 4 + i) * P : (g * 4 + i + 1) * P],
                                identity[:],
                            )
                        k0 = q * (QW // P) + g * 4
                        zt3 = ztbig[:].rearrange("p (k r) -> p k r", r=BLOC)
                        nc.vector.tensor_copy(
                            zt3[:, k0 : k0 + 4, bt * P : (bt + 1) * P],
                            pt[:].rearrange("p (k r) -> p k r", r=P),
                        )
            for bt in range(BT):
                nc.vector.tensor_mul(
                    csb[:, bt : bt + 1],
                    s2col[:, bt : bt + 1],
                    s3col[:, bt : bt + 1],
                )

            # ---- phase C: c -> row -> broadcast across partitions ----
            for bt in range(BT):
                pcc = pmisc.tile([1, P], F32, name="pcc", tag="pmisc")
                nc.tensor.transpose(pcc[:], csb[:, bt : bt + 1], identity[:])
                nc.vector.tensor_copy(crow[0:1, bt * P : (bt + 1) * P], pcc[:])
            for h in range(NH):
                pb = pmisc.tile([P, 512], F32, name="pb", tag="pmisc")
                nc.tensor.matmul(
                    pb[:], ones1[:],
                    crow[0:1, h * 512 : (h + 1) * 512],
                    start=True, stop=True,
                )
                nc.vector.tensor_copy(cbcast[:, h * 512 : (h + 1) * 512], pb[:])

            # ---- phase D: zT = (zT * U1) * c in place (rounds to f32r) ----
            for k in range(KC):
                sl = slice(k * BLOC, (k + 1) * BLOC)
                nc.vector.scalar_tensor_tensor(
                    ztbig[:, sl],
                    ztbig[:, sl],
                    u1sb[:, k : k + 1],
                    cbcast[:],
                    MULT,
                    MULT,
                )

            # ---- phase E: main matmul (float32r), evict, transpose out ----
            wt_view = wt_d[:].rearrange("(k p) o -> p k o", p=P)
            KH = KC // 2
            for ot in range(OT):
                wslabs = []
                for half in range(2):
                    ws = wslabp.tile([P, KH, P], F32R, name="wslab")
                    nc.gpsimd.dma_start(
                        ws[:],
                        wt_view[
                            :, half * KH : (half + 1) * KH, ot * P : (ot + 1) * P
                        ],
                    )
                    wslabs.append(ws)
                psums = [
                    pmain.tile([P, 512], F32, name=f"pm{h}", tag="pmain")
                    for h in range(NH)
                ]
                for k in range(KC):
                    lhs = wslabs[k // KH][:, k % KH, :]
                    for h in range(NH):
                        nc.tensor.matmul(
                            psums[h][:],
                            lhs,
                            ztbig[
                                :, k * BLOC + h * 512 : k * BLOC + (h + 1) * 512
                            ],
                            start=(k == 0),
                            stop=(k == KC - 1),
                        )
                outT = outTp.tile([P, BLOC], F32, name="outT")
                for h in range(NH):
                    nc.vector.tensor_scalar_add(
                        outT[:, h * 512 : (h + 1) * 512],
                        psums[h][:],
                        biassb[:, ot : ot + 1],
                    )
                onat = onatp.tile([P, BLOC], F32, name="onat")
                for g in range(BT // 4):
                    po = ptr.tile([P, 512], F32, name="pt", tag="pt")
                    for i in range(4):
                        bt = g * 4 + i
                        nc.tensor.transpose(
                            po[:, i * P : (i + 1) * P],
                            outT[:, bt * P : (bt + 1) * P],
                            identity[:],
                        )
                    nc.vector.tensor_copy(
                        onat[:, g * 512 : (g + 1) * 512], po[:]
                    )
                nc.gpsimd.dma_start(
                    out_d[:]
                    .rearrange("(t p) o -> p t o", p=P)[
                        :, :, ot * P : (ot + 1) * P
                    ],
                    onat[:].rearrange("p (t o) -> p t o", o=P),
                )

    nc.finalize()
    return nc


_NC_CACHE = {}


def get_nc() -> bass.Bass:
    if "nc" not in _NC_CACHE:
        _NC_CACHE["nc"] = build_nc()
    return _NC_CACHE["nc"]


def kernel(z, U1, U2, U3, W, b):
    from concourse.bass_utils import run_bass_kernel_spmd

    z = np.ascontiguousarray(np.asarray(z, dtype=np.float32)).reshape(B, D)
    U1 = np.asarray(U1, dtype=np.float32)
    U2 = np.asarray(U2, dtype=np.float32)
    U3 = np.asarray(U3, dtype=np.float32)
    W = np.asarray(W, dtype=np.float32)
    bias = np.asarray(b, dtype=np.float32)

    wt = np.ascontiguousarray(W.T)                      # [D, O], layout only
    u23 = np.ascontiguousarray(np.stack([U2, U3], 1))   # [D, 2]

    nc = get_nc()
    in_maps = [
        {
            "z": z[c * BLOC : (c + 1) * BLOC],
            "wt": wt,
            "u1": U1,
            "u23": u23,
            "bias": bias,
        }
        for c in range(NCORES)
    ]
    res = run_bass_kernel_spmd(
        nc,
        in_maps,
        core_ids=list(range(NCORES)),
        trace=bool(int(os.environ.get("KERNEL_TRACE", "0"))),
    )
    if res.exec_time_ns is not None:
        print(f"HW exec time: {res.exec_time_ns} ns", file=sys.stderr)
    kernel.last_results = res
    return np.concatenate([res.results[c]["out"] for c in range(NCORES)], axis=0)
